# revision 25
# baseline (speedup 1.0000x reference)
"""Trainium2 Bass kernel for prefix-LM CausalSelfAttention.

Problem: B=2, T=2048, C=2048, H=16 heads (hd=128), prefix-LM mask
(bidirectional over first half, causal after), RoPE on q/k.

Sharding over 8 cores: data-parallel on batch (2) x tensor-parallel on
heads (4 heads per core). Each core computes a partial output projection
(its heads' contribution); partials are summed on host.

Per-core dataflow:
  A. qT/kT = W^T @ x^T    [hd*4, T] head-major tiles (f32r, full PE rate)
  B. RoPE via pair-swap permutation matmul + DVE combine (f32r)
  C. v = x @ Wv           [T, hd*4] natural layout, cast to bf16
     B and C are emitted interleaved so the PE computes v while the DVE
     does RoPE's elementwise work (PE would otherwise idle ~50us).
  D. Per 512-query chunk I (outer), per head h (inner), over unmasked
     128-key tiles J:
       S'[J] = k_rope[:,J]^T-tile x q_rope[:,I]    (f32r, scores transposed)
       P'[J] = exp(S' * 1/sqrt(hd))                (ACT, PSUM->SBUF, bf16)
       mask-multiply for diagonal-crossing tiles only (DVE bf16)
       y_psum += v[J,h]^T-as-lhsT x P'[J]          (bf16 PV, out y^T [hd,i])
       d_psum += ones^T x P'[J]                    (softmax denominator)
     y^T[:, I] = y_psum * broadcast(1/d)  (recip via fast-approx DVE op;
     the same bf16 P' feeds PV and d so rounding cancels in the ratio)
  E. partial_out = y^T-as-lhsT x Wp (bf16) accumulated over the 4 heads,
     emitted interleaved into stage D one chunk behind (chunk I's proj
     runs while chunk I+1's attention computes); PSUM evacuation on DVE
     (keeps ACT free for exp, which is the stage-D co-bottleneck).

Fully-masked key tiles are skipped (structural sparsity: 44/64 tiles/head).
"""
import math

import numpy as np

N_HEAD = 16
B = 2
T = 2048
C = 2048
HD = 128
HPC = 4          # heads per core
CL = HPC * HD    # local C = 512
TC = 512         # chunk width (matmul moving free dim / psum bank)
NT = T // TC     # 4 chunks
KT = C // 128    # 16 contraction tiles over C
TT = T // 128    # 16 T tiles
SCALE = 1.0 / math.sqrt(HD)

# Per query-chunk I: list of (J, mask_idx) key tiles to compute.
# mask_idx is None for fully-allowed tiles, else 0..3 selecting the
# static diagonal pattern mask[d][jj, ii] = (ii >= jj + 128*d).
_JLISTS = {
    0: [(j, None) for j in range(8)],
    1: [(j, None) for j in range(8)],
    2: [(j, None) for j in range(8)] + [(8 + d, d) for d in range(4)],
    3: [(j, None) for j in range(12)] + [(12 + d, d) for d in range(4)],
}

_CACHE = {}


def _build_nc():
    import concourse.tile as tile
    import concourse.mybir as mybir
    from concourse import bacc
    from concourse import bass_isa

    f32 = mybir.dt.float32
    f32r = mybir.dt.float32r
    bf16 = mybir.dt.bfloat16

    nc = bacc.Bacc(None, target_bir_lowering=False)

    xT = nc.dram_tensor("xT", [C, T], f32r, kind="ExternalInput")
    wqk = nc.dram_tensor("wqk", [C, 2 * CL], f32r, kind="ExternalInput")
    wv = nc.dram_tensor("wv", [C, CL], f32r, kind="ExternalInput")
    wp = nc.dram_tensor("wp", [CL, C], bf16, kind="ExternalInput")
    cosP = nc.dram_tensor("cosP", [HD, T], f32, kind="ExternalInput")
    sinP = nc.dram_tensor("sinP", [HD, T], f32, kind="ExternalInput")
    rt = nc.dram_tensor("rt", [HD, HD], f32r, kind="ExternalInput")
    masks = nc.dram_tensor("masks", [4, 128, TC], bf16, kind="ExternalInput")
    out = nc.dram_tensor("out", [T, C], f32, kind="ExternalOutput")

    xT3 = xT.rearrange("(kt p) t -> p kt t", p=128)
    wqk3 = wqk.rearrange("(kt p) m -> p kt m", p=128)
    wv3 = wv.rearrange("(kt p) m -> p kt m", p=128)
    wp3 = wp.rearrange("(kt p) m -> p kt m", p=128)
    masks3 = masks.rearrange("d p n -> p d n")

    Exp = mybir.ActivationFunctionType.Exp

    with tile.TileContext(nc) as tc:
        mpool = tc.alloc_tile_pool(name="misc", bufs=1)
        qk_pool = tc.alloc_tile_pool(name="qkrope", bufs=1)            # 64K
        tpool = tc.alloc_tile_pool(name="trig", bufs=1, side="right")  # 16K

        rt_sb = mpool.tile([HD, HD], f32r)
        mask_sb = mpool.tile([128, 4, TC], bf16)
        cos_sb = tpool.tile([HD, T], f32)
        sin_sb = tpool.tile([HD, T], f32)

        # qkT[m] for m in 0..7: m<4 -> q head m, else k head m-4; [hd, T]
        # (rope outputs later reuse the same slots via identical tags)
        qkT = [qk_pool.tile([128, T], f32r, tag=f"qk{m}", name=f"qk{m}") for m in range(8)]

        # ---- stage A: qT/kT = W_{q,k}^T @ x^T, head-major tiles ----
        # xpool outlives wpool (reused by stage C) so it sits below it on
        # the pool stack; it is only released in the final LIFO teardown.
        xpool = tc.alloc_tile_pool(name="xt_sb", bufs=1)           # ~48K A+C
        wpool = tc.alloc_tile_pool(name="wqk_sb", bufs=1)          # 64K A
        ps1 = tc.alloc_tile_pool(name="ps_qk", bufs=1, space="PSUM")

        def x_tiles(n):
            ts = []
            for k in range(KT):
                xt = xpool.tile([128, TC], f32r, tag=f"x{k}", name=f"x{k}",
                                bufs=2 if k < 4 else 1)
                nc.sync.dma_start(out=xt, in_=xT3[:, k, n * TC:(n + 1) * TC])
                ts.append(xt)
            return ts

        # The q-half of W (wa) and the first x chunk are DMA'd interleaved,
        # the k-half (wb) after: m-groups 0..3 unlock after 6MB of DMA
        # instead of 12MB, halving the stage-A startup ramp.
        wa_t, wb_t = [], []
        x_first = []
        for k in range(KT):
            wt = wpool.tile([128, CL], f32r, tag=f"wa{k}", name=f"wa{k}")
            nc.sync.dma_start(out=wt, in_=wqk3[:, k, 0:CL])
            wa_t.append(wt)
            xt = xpool.tile([128, TC], f32r, tag=f"x{k}", name=f"x{k}",
                            bufs=2 if k < 4 else 1)
            nc.sync.dma_start(out=xt, in_=xT3[:, k, 0:TC])
            x_first.append(xt)
        for k in range(KT):
            wt = wpool.tile([128, CL], f32r, tag=f"wb{k}", name=f"wb{k}")
            nc.sync.dma_start(out=wt, in_=wqk3[:, k, CL:2 * CL])
            wb_t.append(wt)
        nc.sync.dma_start(out=rt_sb, in_=rt[:, :])
        nc.sync.dma_start(out=cos_sb, in_=cosP[:, :])
        nc.sync.dma_start(out=sin_sb, in_=sinP[:, :])
        for n in range(NT):
            x_t = x_first if n == 0 else x_tiles(n)
            for m in range(8):
                w_t = wa_t if m < 4 else wb_t
                msl = slice((m % 4) * 128, (m % 4 + 1) * 128)
                ps = ps1.tile([128, TC], f32, tag="ps_qk", name="ps_qk", bufs=8)
                for k in range(KT):
                    nc.tensor.matmul(
                        ps, w_t[k][:, msl], x_t[k],
                        start=(k == 0), stop=(k == KT - 1),
                    )
                nc.vector.tensor_copy(out=qkT[m][:, n * TC:(n + 1) * TC], in_=ps)
        wpool.release()
        ps1.release()

        # ---- stages B+C interleaved: RoPE (DVE-heavy) + v (PE-heavy) ----
        # v-phase weights and the re-streamed x chunks land in the space
        # wqk_sb/xt_sb just released; the fresh xb pool decouples the
        # re-stream DMAs from stage A's tile lifetimes so they prefetch.
        v_pool = tc.alloc_tile_pool(name="v_sb", bufs=1)           # 16K
        wppool = tc.alloc_tile_pool(name="wp_sb", bufs=1)          # 16K
        wvpool = tc.alloc_tile_pool(name="wv_sb", bufs=1)          # 32K
        v_t = [v_pool.tile([128, CL], bf16, tag=f"v{mt}", name=f"v{mt}")
               for mt in range(TT)]
        wv_t = []
        for k in range(KT):
            wt = wvpool.tile([128, CL], f32r, tag=f"wv{k}", name=f"wv{k}")
            nc.sync.dma_start(out=wt, in_=wv3[:, k])
            wv_t.append(wt)
        nc.sync.dma_start(out=mask_sb, in_=masks3)

        rope = [None] * 8
        rtmp = tc.alloc_tile_pool(name="rope_tmp", bufs=4)
        psr = tc.alloc_tile_pool(name="ps_rot", bufs=2, space="PSUM")
        ps2 = tc.alloc_tile_pool(name="ps_v", bufs=2, space="PSUM")
        m_order = (0, 4, 1, 5, 2, 6, 3, 7)

        # RoPE is emitted piecewise (one 512-col chunk at a time) between
        # v-matmul groups, so the PE keeps streaming v while the DVE
        # digests the elementwise work; v PSUM evacuation runs on the
        # otherwise-idle ACT engine.
        rope_state = {}  # m -> list of (t1, t2)

        def rope_piece(m, nn):
            sl = slice(nn * TC, (nn + 1) * TC)
            ps = psr.tile([128, TC], f32, tag="ps_rot", name="ps_rot")
            nc.tensor.matmul(ps, rt_sb, qkT[m][:, sl], start=True, stop=True)
            t1 = rtmp.tile([128, TC], f32, tag="t1", name="t1")
            t2 = rtmp.tile([128, TC], f32, tag="t2", name="t2")
            nc.vector.tensor_mul(t1, ps, sin_sb[:, sl])
            nc.vector.tensor_mul(t2, qkT[m][:, sl], cos_sb[:, sl])
            rope_state.setdefault(m, []).append((t1, t2))
            if nn == NT - 1:
                # all reads of qkT[m] issued; now write into its slot
                ro = qk_pool.tile([128, T], f32r, tag=f"qk{m}", name=f"rope{m}")
                for k in range(NT):
                    ksl = slice(k * TC, (k + 1) * TC)
                    ta, tb = rope_state[m][k]
                    nc.vector.tensor_add(ro[:, ksl], ta, tb)
                rope[m] = ro

        rope_pieces = [(m, nn) for m in m_order for nn in range(NT)]
        rp = 0
        # chunk 3's x tiles are still resident from stage A — consume them
        # first (no DMA wait), re-streaming chunks 0..2 behind them.
        for n, x_t in [(3, x_t), (0, None), (1, None), (2, None)]:
            if x_t is None:
                x_t = x_tiles(n)
            for sub in range(4):
                mt = 4 * n + sub
                ps = ps2.tile([128, CL], f32, tag="ps_v", name="ps_v")
                for k in range(KT):
                    nc.tensor.matmul(
                        ps, x_t[k][:, sub * 128:(sub + 1) * 128], wv_t[k],
                        start=(k == 0), stop=(k == KT - 1),
                    )
                nc.scalar.copy(out=v_t[mt], in_=ps)
                for _ in range(2):
                    if rp < len(rope_pieces):
                        rope_piece(*rope_pieces[rp])
                        rp += 1
        while rp < len(rope_pieces):
            rope_piece(*rope_pieces[rp])
            rp += 1
        # output-projection weights for stage E
        wp_t = []
        for hk in range(HPC):
            wt = wppool.tile([128, C], bf16, tag=f"wp{hk}", name=f"wp{hk}")
            nc.sync.dma_start(out=wt, in_=wp3[:, hk])
            wp_t.append(wt)
        rtmp.release()
        wvpool.release()
        ps2.release()
        psr.release()
        tpool.release()

        # ---- stage D (I-outer, h-inner) with stage E one chunk behind ----
        y_pool = tc.alloc_tile_pool(name="yT_sb", bufs=1)          # 16K
        yT = [y_pool.tile([128, T], bf16, tag=f"yT{h}", name=f"yT{h}")
              for h in range(HPC)]

        pp_pool = tc.alloc_tile_pool(name="pp", bufs=4)
        sm_pool = tc.alloc_tile_pool(name="small", bufs=2)
        opool = tc.alloc_tile_pool(name="ostage", bufs=3)
        # 8 PSUM banks: 4 score (deep exp lookahead) + 2 y + 2 proj.
        # The softmax denominator no longer uses the PE/PSUM at all: it is
        # accumulated from the bf16 P' tiles by GpSimd pair-adds + a DVE
        # chain, partition-all-reduced on GpSimd, and applied with a single
        # DVE divide.
        ps_s = tc.alloc_tile_pool(name="ps_s", bufs=4, space="PSUM")
        ps_y = tc.alloc_tile_pool(name="ps_y", bufs=2, space="PSUM")
        ps_o = tc.alloc_tile_pool(name="ps_o", bufs=2, space="PSUM")

        e_queue = []

        def emit_e_group():
            mt, nn = e_queue.pop(0)
            msl = slice(mt * 128, (mt + 1) * 128)
            ps = ps_o.tile([128, TC], f32, tag="o", name="o_ps")
            for hk in range(HPC):
                nc.tensor.matmul(
                    ps, yT[hk][:, msl], wp_t[hk][:, nn * TC:(nn + 1) * TC],
                    start=(hk == 0), stop=(hk == HPC - 1),
                )
            ot = opool.tile([128, TC], f32, tag="ot", name="ot")
            nc.vector.tensor_copy(out=ot, in_=ps)
            nc.sync.dma_start(out=out[msl, nn * TC:(nn + 1) * TC], in_=ot)

        # Stage D processes key tiles in PAIRS with a two-pair S lookahead:
        # the PE stream is [S S] [E] [PV PV] [d d] per pair. Consecutive
        # same-bank matmuls (PV,PV / d,d) avoid the ~107ns PSUM
        # bank-switch drain penalty, and the 4-tile-deep exp pipeline plus
        # the woven stage-E group give the Scalar engine's exp a full
        # pair-period of slack before PV consumes it.
        for I in range(NT):
            isl = slice(I * TC, (I + 1) * TC)
            jl = _JLISTS[I]
            pairs = [jl[i:i + 2] for i in range(0, len(jl), 2)]
            for h in range(HPC):
                q_h = rope[h]
                k_h = rope[4 + h]
                y_ps = ps_y.tile([128, TC], f32, tag="y", name="y_ps")

                def emit_s_pair(pair):
                    pps = []
                    for (J, dm) in pair:
                        s_ps = ps_s.tile([128, TC], f32, tag="s", name="s_ps")
                        nc.tensor.matmul(
                            s_ps, k_h[:, J * 128:(J + 1) * 128],
                            q_h[:, isl], start=True, stop=True,
                        )
                        pp = pp_pool.tile([128, TC], bf16, tag="pp", name="pp",
                                          bufs=6)
                        nc.scalar.activation(out=pp, in_=s_ps, func=Exp,
                                             scale=SCALE)
                        pps.append(pp)
                    return pps

                pending = [emit_s_pair(pairs[0])]
                if e_queue:
                    emit_e_group()
                if len(pairs) > 1:
                    pending.append(emit_s_pair(pairs[1]))
                jidx = 0
                dacc = sm_pool.tile([128, TC], f32, tag="dacc", name="dacc")
                for t, pair in enumerate(pairs):
                    pps = pending.pop(0)
                    if t + 2 < len(pairs):
                        pending.append(emit_s_pair(pairs[t + 2]))
                    if e_queue and (t % 2 == 1 or I == NT - 1):
                        emit_e_group()
                    for pi, (J, dm) in enumerate(pair):
                        if dm is not None:
                            ppm = pp_pool.tile([128, TC], bf16, tag="ppm",
                                               name="ppm", bufs=2)
                            nc.vector.tensor_mul(ppm, pps[pi], mask_sb[:, dm])
                            pps[pi] = ppm
                    tpair = sm_pool.tile([128, TC], f32, tag="tpair",
                                         name="tpair", bufs=3)
                    nc.gpsimd.tensor_add(tpair, pps[0], pps[1])
                    for pi, (J, dm) in enumerate(pair):
                        nc.tensor.matmul(
                            y_ps, v_t[J][:, h * 128:(h + 1) * 128], pps[pi],
                            start=(jidx + pi == 0),
                            stop=(jidx + pi == len(jl) - 1),
                        )
                    if t == 0:
                        nc.vector.tensor_copy(out=dacc, in_=tpair)
                    else:
                        nc.vector.tensor_add(dacc, dacc, tpair)
                    jidx += len(pair)
                dall = sm_pool.tile([128, TC], f32, tag="dall", name="dall")
                nc.gpsimd.partition_all_reduce(
                    dall, dacc, 128, bass_isa.ReduceOp.add)
                recipB = sm_pool.tile([128, TC], f32, tag="recipB",
                                      name="recipB")
                nc.vector.reciprocal_approx_fast(out=recipB, in_=dall)
                nc.vector.tensor_mul(yT[h][:, isl], y_ps, recipB)
            e_queue.extend((4 * I + ml, nn) for ml in range(4) for nn in range(NT))
        while e_queue:
            emit_e_group()

        for p in (opool, sm_pool, pp_pool, y_pool, wppool, v_pool,
                  xpool, qk_pool, mpool, ps_o, ps_y, ps_s):
            p.release()
    nc.compile()
    return nc


def _host_prep(x, w_qkv, w_proj, freqs_cis):
    """Build per-core input maps (slicing + layout prep only)."""
    import ml_dtypes

    bf16 = ml_dtypes.bfloat16
    x = np.asarray(x, dtype=np.float32)
    w_qkv = np.asarray(w_qkv, dtype=np.float32)
    w_proj = np.asarray(w_proj, dtype=np.float32)
    fc = np.asarray(freqs_cis, dtype=np.float32)

    xTb = [np.ascontiguousarray(x[b].T) for b in range(B)]

    cos = fc[:, :, 0].T  # [64, T]
    sin = fc[:, :, 1].T
    cosP = np.repeat(cos, 2, axis=0).astype(np.float32)  # [128, T]
    sinP = np.repeat(sin, 2, axis=0).astype(np.float32)

    rt = np.zeros((HD, HD), dtype=np.float32)
    for d in range(HD // 2):
        rt[2 * d, 2 * d + 1] = 1.0
        rt[2 * d + 1, 2 * d] = -1.0

    masks = np.zeros((4, 128, TC), dtype=np.float32)
    ii = np.arange(TC)[None, :]
    jj = np.arange(128)[:, None]
    for d in range(4):
        masks[d] = (ii >= jj + 128 * d).astype(np.float32)
    masks = masks.astype(bf16)

    in_maps = []
    for core in range(8):
        b = core // 4
        g = core % 4
        qc = np.ascontiguousarray(w_qkv[:, 512 * g: 512 * (g + 1)])
        kc = np.ascontiguousarray(w_qkv[:, 2048 + 512 * g: 2048 + 512 * (g + 1)])
        vc = np.ascontiguousarray(w_qkv[:, 4096 + 512 * g: 4096 + 512 * (g + 1)])
        wqk_c = np.concatenate([qc, kc], axis=1)
        wp_c = np.ascontiguousarray(
            w_proj[512 * g: 512 * (g + 1), :]).astype(bf16)
        in_maps.append({
            "xT": xTb[b],
            "wqk": wqk_c,
            "wv": vc,
            "wp": wp_c,
            "cosP": cosP,
            "sinP": sinP,
            "rt": rt,
            "masks": masks,
        })
    return in_maps


def _get_nc():
    if "nc" not in _CACHE:
        _CACHE["nc"] = _build_nc()
    return _CACHE["nc"]


def kernel(x, w_qkv, w_proj, freqs_cis, attn_mask, _trace=False):
    from concourse.bass_utils import run_bass_kernel_spmd

    in_maps = _host_prep(x, w_qkv, w_proj, freqs_cis)
    nc = _get_nc()
    res = run_bass_kernel_spmd(
        nc, in_maps, core_ids=list(range(8)), trace=_trace,
    )
    outs = [r["out"].astype(np.float64) for r in res.results]
    full = np.stack([
        outs[0] + outs[1] + outs[2] + outs[3],
        outs[4] + outs[5] + outs[6] + outs[7],
    ]).astype(np.float32)
    if _trace:
        kernel._last_results = res
    return full


# revision 27
# speedup vs baseline: 1.5571x; 1.5571x over previous
"""Trainium2 Bass kernel for prefix-LM CausalSelfAttention.

Problem: B=2, T=2048, C=2048, H=16 heads (hd=128), prefix-LM mask
(bidirectional over first half, causal after), RoPE on q/k.

Sharding over 8 cores: data-parallel on batch (2) x tensor-parallel on
heads (4 heads per core). Each core computes a partial output projection
(its heads' contribution); partials are summed on host.

Per-core dataflow:
  A. qT/kT = W^T @ x^T    [hd*4, T] head-major tiles (f32r, full PE rate)
  B. RoPE via pair-swap permutation matmul + DVE combine (f32r)
  C. v = x @ Wv           [T, hd*4] natural layout, cast to bf16
     B and C are emitted interleaved so the PE computes v while the DVE
     does RoPE's elementwise work (PE would otherwise idle ~50us).
  D. Per 512-query chunk I (outer), per head h (inner), over unmasked
     128-key tiles J:
       S'[J] = k_rope[:,J]^T-tile x q_rope[:,I]    (f32r, scores transposed)
       P'[J] = exp(S' * 1/sqrt(hd))                (ACT, PSUM->SBUF, bf16)
       mask-multiply for diagonal-crossing tiles only (DVE bf16)
       y_psum += v[J,h]^T-as-lhsT x P'[J]          (bf16 PV, out y^T [hd,i])
       d_psum += ones^T x P'[J]                    (softmax denominator)
     y^T[:, I] = y_psum * broadcast(1/d)  (recip via fast-approx DVE op;
     the same bf16 P' feeds PV and d so rounding cancels in the ratio)
  E. partial_out = y^T-as-lhsT x Wp (bf16) accumulated over the 4 heads,
     emitted interleaved into stage D one chunk behind (chunk I's proj
     runs while chunk I+1's attention computes); PSUM evacuation on DVE
     (keeps ACT free for exp, which is the stage-D co-bottleneck).

Fully-masked key tiles are skipped (structural sparsity: 44/64 tiles/head).
"""
import math

import numpy as np

N_HEAD = 16
B = 2
T = 2048
C = 2048
HD = 128
HPC = 4          # heads per core
CL = HPC * HD    # local C = 512
TC = 512         # chunk width (matmul moving free dim / psum bank)
NT = T // TC     # 4 chunks
KT = C // 128    # 16 contraction tiles over C
TT = T // 128    # 16 T tiles
SCALE = 1.0 / math.sqrt(HD)

# Per query-chunk I: list of (J, mask_idx) key tiles to compute.
# mask_idx is None for fully-allowed tiles, else 0..3 selecting the
# static diagonal pattern mask[d][jj, ii] = (ii >= jj + 128*d).
_JLISTS = {
    0: [(j, None) for j in range(8)],
    1: [(j, None) for j in range(8)],
    2: [(j, None) for j in range(8)] + [(8 + d, d) for d in range(4)],
    3: [(j, None) for j in range(12)] + [(12 + d, d) for d in range(4)],
}

_CACHE = {}


def _build_nc():
    import concourse.tile as tile
    import concourse.mybir as mybir
    from concourse import bacc
    from concourse import bass_isa

    f32 = mybir.dt.float32
    f32r = mybir.dt.float32r
    bf16 = mybir.dt.bfloat16

    nc = bacc.Bacc(None, target_bir_lowering=False)

    xT = nc.dram_tensor("xT", [C, T], f32r, kind="ExternalInput")
    wqk = nc.dram_tensor("wqk", [C, 2 * CL], f32r, kind="ExternalInput")
    wv = nc.dram_tensor("wv", [C, CL], f32r, kind="ExternalInput")
    wp = nc.dram_tensor("wp", [CL, C], bf16, kind="ExternalInput")
    cosP = nc.dram_tensor("cosP", [HD, T], f32, kind="ExternalInput")
    sinP = nc.dram_tensor("sinP", [HD, T], f32, kind="ExternalInput")
    rt = nc.dram_tensor("rt", [HD, HD], f32r, kind="ExternalInput")
    masks = nc.dram_tensor("masks", [4, 128, TC], bf16, kind="ExternalInput")
    ones = nc.dram_tensor("ones", [128, 128], bf16, kind="ExternalInput")
    out = nc.dram_tensor("out", [T, C], f32, kind="ExternalOutput")

    xT3 = xT.rearrange("(kt p) t -> p kt t", p=128)
    wqk3 = wqk.rearrange("(kt p) m -> p kt m", p=128)
    wv3 = wv.rearrange("(kt p) m -> p kt m", p=128)
    wp3 = wp.rearrange("(kt p) m -> p kt m", p=128)
    masks3 = masks.rearrange("d p n -> p d n")

    Exp = mybir.ActivationFunctionType.Exp

    with tile.TileContext(nc) as tc:
        mpool = tc.alloc_tile_pool(name="misc", bufs=1)
        qk_pool = tc.alloc_tile_pool(name="qkrope", bufs=1)            # 64K
        tpool = tc.alloc_tile_pool(name="trig", bufs=1, side="right")  # 16K

        rt_sb = mpool.tile([HD, HD], f32r)
        ones_sb = mpool.tile([128, 128], bf16)
        mask_sb = mpool.tile([128, 4, TC], bf16)
        cos_sb = tpool.tile([HD, T], f32)
        sin_sb = tpool.tile([HD, T], f32)

        # qkT[m] for m in 0..7: m<4 -> q head m, else k head m-4; [hd, T]
        # (rope outputs later reuse the same slots via identical tags)
        qkT = [qk_pool.tile([128, T], f32r, tag=f"qk{m}", name=f"qk{m}") for m in range(8)]

        # ---- stage A: qT/kT = W_{q,k}^T @ x^T, head-major tiles ----
        # xpool outlives wpool (reused by stage C) so it sits below it on
        # the pool stack; it is only released in the final LIFO teardown.
        xpool = tc.alloc_tile_pool(name="xt_sb", bufs=1)           # ~48K A+C
        wpool = tc.alloc_tile_pool(name="wqk_sb", bufs=1)          # 64K A
        ps1 = tc.alloc_tile_pool(name="ps_qk", bufs=1, space="PSUM")

        def x_tiles(n):
            ts = []
            for k in range(KT):
                xt = xpool.tile([128, TC], f32r, tag=f"x{k}", name=f"x{k}",
                                bufs=2 if k < 4 else 1)
                nc.sync.dma_start(out=xt, in_=xT3[:, k, n * TC:(n + 1) * TC])
                ts.append(xt)
            return ts

        # The q-half of W (wa) and the first x chunk are DMA'd interleaved,
        # the k-half (wb) after: m-groups 0..3 unlock after 6MB of DMA
        # instead of 12MB, halving the stage-A startup ramp.
        wa_t, wb_t = [], []
        x_first = []
        for k in range(KT):
            wt = wpool.tile([128, CL], f32r, tag=f"wa{k}", name=f"wa{k}")
            nc.sync.dma_start(out=wt, in_=wqk3[:, k, 0:CL])
            wa_t.append(wt)
            xt = xpool.tile([128, TC], f32r, tag=f"x{k}", name=f"x{k}",
                            bufs=2 if k < 4 else 1)
            nc.sync.dma_start(out=xt, in_=xT3[:, k, 0:TC])
            x_first.append(xt)
        for k in range(KT):
            wt = wpool.tile([128, CL], f32r, tag=f"wb{k}", name=f"wb{k}")
            nc.sync.dma_start(out=wt, in_=wqk3[:, k, CL:2 * CL])
            wb_t.append(wt)
        nc.sync.dma_start(out=rt_sb, in_=rt[:, :])
        nc.sync.dma_start(out=cos_sb, in_=cosP[:, :])
        nc.sync.dma_start(out=sin_sb, in_=sinP[:, :])
        for n in range(NT):
            x_t = x_first if n == 0 else x_tiles(n)
            for m in range(8):
                w_t = wa_t if m < 4 else wb_t
                msl = slice((m % 4) * 128, (m % 4 + 1) * 128)
                ps = ps1.tile([128, TC], f32, tag="ps_qk", name="ps_qk", bufs=8)
                for k in range(KT):
                    nc.tensor.matmul(
                        ps, w_t[k][:, msl], x_t[k],
                        start=(k == 0), stop=(k == KT - 1),
                    )
                nc.vector.tensor_copy(out=qkT[m][:, n * TC:(n + 1) * TC], in_=ps)
        wpool.release()
        ps1.release()

        # ---- stages B+C interleaved: RoPE (DVE-heavy) + v (PE-heavy) ----
        # v-phase weights and the re-streamed x chunks land in the space
        # wqk_sb/xt_sb just released; the fresh xb pool decouples the
        # re-stream DMAs from stage A's tile lifetimes so they prefetch.
        v_pool = tc.alloc_tile_pool(name="v_sb", bufs=1)           # 16K
        wppool = tc.alloc_tile_pool(name="wp_sb", bufs=1)          # 16K
        wvpool = tc.alloc_tile_pool(name="wv_sb", bufs=1)          # 32K
        v_t = [v_pool.tile([128, CL], bf16, tag=f"v{mt}", name=f"v{mt}")
               for mt in range(TT)]
        wv_t = []
        for k in range(KT):
            wt = wvpool.tile([128, CL], f32r, tag=f"wv{k}", name=f"wv{k}")
            nc.sync.dma_start(out=wt, in_=wv3[:, k])
            wv_t.append(wt)
        nc.sync.dma_start(out=ones_sb, in_=ones[:, :])
        nc.sync.dma_start(out=mask_sb, in_=masks3)

        rope = [None] * 8
        rtmp = tc.alloc_tile_pool(name="rope_tmp", bufs=4)
        psr = tc.alloc_tile_pool(name="ps_rot", bufs=2, space="PSUM")
        ps2 = tc.alloc_tile_pool(name="ps_v", bufs=2, space="PSUM")
        m_order = (0, 4, 1, 5, 2, 6, 3, 7)

        # RoPE is emitted piecewise (one 512-col chunk at a time) between
        # v-matmul groups, so the PE keeps streaming v while the DVE
        # digests the elementwise work; v PSUM evacuation runs on the
        # otherwise-idle ACT engine.
        rope_state = {}  # m -> list of (t1, t2)

        def rope_piece(m, nn):
            sl = slice(nn * TC, (nn + 1) * TC)
            ps = psr.tile([128, TC], f32, tag="ps_rot", name="ps_rot")
            nc.tensor.matmul(ps, rt_sb, qkT[m][:, sl], start=True, stop=True)
            t1 = rtmp.tile([128, TC], f32, tag="t1", name="t1")
            t2 = rtmp.tile([128, TC], f32, tag="t2", name="t2")
            nc.vector.tensor_mul(t1, ps, sin_sb[:, sl])
            nc.vector.tensor_mul(t2, qkT[m][:, sl], cos_sb[:, sl])
            rope_state.setdefault(m, []).append((t1, t2))
            if nn == NT - 1:
                # all reads of qkT[m] issued; now write into its slot
                ro = qk_pool.tile([128, T], f32r, tag=f"qk{m}", name=f"rope{m}")
                for k in range(NT):
                    ksl = slice(k * TC, (k + 1) * TC)
                    ta, tb = rope_state[m][k]
                    nc.vector.tensor_add(ro[:, ksl], ta, tb)
                rope[m] = ro

        rope_pieces = [(m, nn) for m in m_order for nn in range(NT)]
        rp = 0
        # chunk 3's x tiles are still resident from stage A — consume them
        # first (no DMA wait), re-streaming chunks 0..2 behind them.
        for n, x_t in [(3, x_t), (0, None), (1, None), (2, None)]:
            if x_t is None:
                x_t = x_tiles(n)
            for sub in range(4):
                mt = 4 * n + sub
                ps = ps2.tile([128, CL], f32, tag="ps_v", name="ps_v")
                for k in range(KT):
                    nc.tensor.matmul(
                        ps, x_t[k][:, sub * 128:(sub + 1) * 128], wv_t[k],
                        start=(k == 0), stop=(k == KT - 1),
                    )
                nc.scalar.copy(out=v_t[mt], in_=ps)
                for _ in range(2):
                    if rp < len(rope_pieces):
                        rope_piece(*rope_pieces[rp])
                        rp += 1
        while rp < len(rope_pieces):
            rope_piece(*rope_pieces[rp])
            rp += 1
        # output-projection weights for stage E
        wp_t = []
        for hk in range(HPC):
            wt = wppool.tile([128, C], bf16, tag=f"wp{hk}", name=f"wp{hk}")
            nc.sync.dma_start(out=wt, in_=wp3[:, hk])
            wp_t.append(wt)
        rtmp.release()
        wvpool.release()
        ps2.release()
        psr.release()
        tpool.release()

        # ---- stage D (I-outer, h-inner) with stage E one chunk behind ----
        y_pool = tc.alloc_tile_pool(name="yT_sb", bufs=1)          # 16K
        yT = [y_pool.tile([128, T], bf16, tag=f"yT{h}", name=f"yT{h}")
              for h in range(HPC)]

        pp_pool = tc.alloc_tile_pool(name="pp", bufs=4)
        sm_pool = tc.alloc_tile_pool(name="small", bufs=2)
        opool = tc.alloc_tile_pool(name="ostage", bufs=3)
        # 8 PSUM banks: 3 score (2-deep exp lookahead) + 2 y + 1 d + 2
        # proj. ps_d single-buffered: the reciprocal reads it within ~1us
        # and the next head's first d-matmul lands later than that. The
        # d-matmul's stationary operand is an all-ones [128,128] matrix, so
        # d lands replicated across all partitions and the reciprocal feeds
        # the normalize multiply directly (no partition broadcast).
        ps_s = tc.alloc_tile_pool(name="ps_s", bufs=3, space="PSUM")
        ps_y = tc.alloc_tile_pool(name="ps_y", bufs=2, space="PSUM")
        ps_d = tc.alloc_tile_pool(name="ps_d", bufs=1, space="PSUM")
        ps_o = tc.alloc_tile_pool(name="ps_o", bufs=2, space="PSUM")

        e_queue = []

        def emit_e_group():
            mt, nn = e_queue.pop(0)
            msl = slice(mt * 128, (mt + 1) * 128)
            ps = ps_o.tile([128, TC], f32, tag="o", name="o_ps")
            for hk in range(HPC):
                nc.tensor.matmul(
                    ps, yT[hk][:, msl], wp_t[hk][:, nn * TC:(nn + 1) * TC],
                    start=(hk == 0), stop=(hk == HPC - 1),
                )
            ot = opool.tile([128, TC], f32, tag="ot", name="ot")
            nc.vector.tensor_copy(out=ot, in_=ps)
            nc.sync.dma_start(out=out[msl, nn * TC:(nn + 1) * TC], in_=ot)

        # Stage D processes key tiles in PAIRS with a two-pair S lookahead:
        # the PE stream is [S S] [E] [PV PV] [d d] per pair. Consecutive
        # same-bank matmuls (PV,PV / d,d) avoid the ~107ns PSUM
        # bank-switch drain penalty, and the 4-tile-deep exp pipeline plus
        # the woven stage-E group give the Scalar engine's exp a full
        # pair-period of slack before PV consumes it.
        for I in range(NT):
            isl = slice(I * TC, (I + 1) * TC)
            jl = _JLISTS[I]
            pairs = [jl[i:i + 2] for i in range(0, len(jl), 2)]
            for h in range(HPC):
                q_h = rope[h]
                k_h = rope[4 + h]
                y_ps = ps_y.tile([128, TC], f32, tag="y", name="y_ps")
                d_ps = ps_d.tile([128, TC], f32, tag="d", name="d_ps")

                def emit_s_pair(pair):
                    pps = []
                    for (J, dm) in pair:
                        s_ps = ps_s.tile([128, TC], f32, tag="s", name="s_ps")
                        nc.tensor.matmul(
                            s_ps, k_h[:, J * 128:(J + 1) * 128],
                            q_h[:, isl], start=True, stop=True,
                        )
                        pp = pp_pool.tile([128, TC], bf16, tag="pp", name="pp",
                                          bufs=6)
                        nc.scalar.activation(out=pp, in_=s_ps, func=Exp,
                                             scale=SCALE)
                        pps.append(pp)
                    return pps

                pending = [emit_s_pair(pairs[0])]
                if e_queue:
                    emit_e_group()
                if len(pairs) > 1:
                    pending.append(emit_s_pair(pairs[1]))
                jidx = 0
                for t, pair in enumerate(pairs):
                    pps = pending.pop(0)
                    if t + 2 < len(pairs):
                        pending.append(emit_s_pair(pairs[t + 2]))
                    if e_queue and (t % 2 == 1 or I == NT - 1):
                        emit_e_group()
                    for pi, (J, dm) in enumerate(pair):
                        if dm is not None:
                            ppm = pp_pool.tile([128, TC], bf16, tag="ppm",
                                               name="ppm", bufs=2)
                            nc.vector.tensor_mul(ppm, pps[pi], mask_sb[:, dm])
                            pps[pi] = ppm
                    for pi, (J, dm) in enumerate(pair):
                        nc.tensor.matmul(
                            y_ps, v_t[J][:, h * 128:(h + 1) * 128], pps[pi],
                            start=(jidx + pi == 0),
                            stop=(jidx + pi == len(jl) - 1),
                        )
                    for pi, (J, dm) in enumerate(pair):
                        nc.tensor.matmul(d_ps, ones_sb, pps[pi],
                                         start=(jidx + pi == 0),
                                         stop=(jidx + pi == len(jl) - 1))
                    jidx += len(pair)
                recipB = sm_pool.tile([128, TC], f32, tag="recipB",
                                      name="recipB")
                nc.vector.reciprocal_approx_fast(out=recipB, in_=d_ps)
                nc.vector.tensor_mul(yT[h][:, isl], y_ps, recipB)
            e_queue.extend((4 * I + ml, nn) for ml in range(4) for nn in range(NT))
        while e_queue:
            emit_e_group()

        for p in (opool, sm_pool, pp_pool, y_pool, wppool, v_pool,
                  xpool, qk_pool, mpool, ps_o, ps_d, ps_y, ps_s):
            p.release()
    nc.compile()
    return nc


def _host_prep(x, w_qkv, w_proj, freqs_cis):
    """Build per-core input maps (slicing + layout prep only)."""
    import ml_dtypes

    bf16 = ml_dtypes.bfloat16
    x = np.asarray(x, dtype=np.float32)
    w_qkv = np.asarray(w_qkv, dtype=np.float32)
    w_proj = np.asarray(w_proj, dtype=np.float32)
    fc = np.asarray(freqs_cis, dtype=np.float32)

    xTb = [np.ascontiguousarray(x[b].T) for b in range(B)]

    cos = fc[:, :, 0].T  # [64, T]
    sin = fc[:, :, 1].T
    cosP = np.repeat(cos, 2, axis=0).astype(np.float32)  # [128, T]
    sinP = np.repeat(sin, 2, axis=0).astype(np.float32)

    rt = np.zeros((HD, HD), dtype=np.float32)
    for d in range(HD // 2):
        rt[2 * d, 2 * d + 1] = 1.0
        rt[2 * d + 1, 2 * d] = -1.0

    masks = np.zeros((4, 128, TC), dtype=np.float32)
    ii = np.arange(TC)[None, :]
    jj = np.arange(128)[:, None]
    for d in range(4):
        masks[d] = (ii >= jj + 128 * d).astype(np.float32)
    masks = masks.astype(bf16)

    ones = np.ones((128, 128), dtype=bf16)

    in_maps = []
    for core in range(8):
        b = core // 4
        g = core % 4
        qc = np.ascontiguousarray(w_qkv[:, 512 * g: 512 * (g + 1)])
        kc = np.ascontiguousarray(w_qkv[:, 2048 + 512 * g: 2048 + 512 * (g + 1)])
        vc = np.ascontiguousarray(w_qkv[:, 4096 + 512 * g: 4096 + 512 * (g + 1)])
        wqk_c = np.concatenate([qc, kc], axis=1)
        wp_c = np.ascontiguousarray(
            w_proj[512 * g: 512 * (g + 1), :]).astype(bf16)
        in_maps.append({
            "xT": xTb[b],
            "wqk": wqk_c,
            "wv": vc,
            "wp": wp_c,
            "cosP": cosP,
            "sinP": sinP,
            "rt": rt,
            "masks": masks,
            "ones": ones,
        })
    return in_maps


def _get_nc():
    if "nc" not in _CACHE:
        _CACHE["nc"] = _build_nc()
    return _CACHE["nc"]


def kernel(x, w_qkv, w_proj, freqs_cis, attn_mask, _trace=False):
    from concourse.bass_utils import run_bass_kernel_spmd

    in_maps = _host_prep(x, w_qkv, w_proj, freqs_cis)
    nc = _get_nc()
    res = run_bass_kernel_spmd(
        nc, in_maps, core_ids=list(range(8)), trace=_trace,
    )
    outs = [r["out"].astype(np.float64) for r in res.results]
    full = np.stack([
        outs[0] + outs[1] + outs[2] + outs[3],
        outs[4] + outs[5] + outs[6] + outs[7],
    ]).astype(np.float32)
    if _trace:
        kernel._last_results = res
    return full


# revision 28
# speedup vs baseline: 1.5982x; 1.0264x over previous
"""Trainium2 Bass kernel for prefix-LM CausalSelfAttention.

Problem: B=2, T=2048, C=2048, H=16 heads (hd=128), prefix-LM mask
(bidirectional over first half, causal after), RoPE on q/k.

Sharding over 8 cores: data-parallel on batch (2) x tensor-parallel on
heads (4 heads per core). Each core computes a partial output projection
(its heads' contribution); partials are summed on host.

Per-core dataflow:
  A. qT/kT = W^T @ x^T    [hd*4, T] head-major tiles (f32r, full PE rate)
  B. RoPE via pair-swap permutation matmul + DVE combine (f32r)
  C. v = x @ Wv           [T, hd*4] natural layout, cast to bf16
     B and C are emitted interleaved so the PE computes v while the DVE
     does RoPE's elementwise work (PE would otherwise idle ~50us).
  D. Per 512-query chunk I (outer), per head h (inner), over unmasked
     128-key tiles J:
       S'[J] = k_rope[:,J]^T-tile x q_rope[:,I]    (f32r, scores transposed)
       P'[J] = exp(S' * 1/sqrt(hd))                (ACT, PSUM->SBUF, bf16)
       mask-multiply for diagonal-crossing tiles only (DVE bf16)
       y_psum += v[J,h]^T-as-lhsT x P'[J]          (bf16 PV, out y^T [hd,i])
       d_psum += ones^T x P'[J]                    (softmax denominator)
     y^T[:, I] = y_psum * broadcast(1/d)  (recip via fast-approx DVE op;
     the same bf16 P' feeds PV and d so rounding cancels in the ratio)
  E. partial_out = y^T-as-lhsT x Wp (bf16) accumulated over the 4 heads,
     emitted interleaved into stage D one chunk behind (chunk I's proj
     runs while chunk I+1's attention computes); PSUM evacuation on DVE
     (keeps ACT free for exp, which is the stage-D co-bottleneck).

Fully-masked key tiles are skipped (structural sparsity: 44/64 tiles/head).
"""
import math

import numpy as np

N_HEAD = 16
B = 2
T = 2048
C = 2048
HD = 128
HPC = 4          # heads per core
CL = HPC * HD    # local C = 512
TC = 512         # chunk width (matmul moving free dim / psum bank)
NT = T // TC     # 4 chunks
KT = C // 128    # 16 contraction tiles over C
TT = T // 128    # 16 T tiles
SCALE = 1.0 / math.sqrt(HD)

# Per query-chunk I: list of (J, mask_idx) key tiles to compute.
# mask_idx is None for fully-allowed tiles, else 0..3 selecting the
# static diagonal pattern mask[d][jj, ii] = (ii >= jj + 128*d).
_JLISTS = {
    0: [(j, None) for j in range(8)],
    1: [(j, None) for j in range(8)],
    2: [(j, None) for j in range(8)] + [(8 + d, d) for d in range(4)],
    3: [(j, None) for j in range(12)] + [(12 + d, d) for d in range(4)],
}

_CACHE = {}


def _build_nc():
    import concourse.tile as tile
    import concourse.mybir as mybir
    from concourse import bacc
    from concourse import bass_isa

    f32 = mybir.dt.float32
    f32r = mybir.dt.float32r
    bf16 = mybir.dt.bfloat16

    nc = bacc.Bacc(None, target_bir_lowering=False)

    xT = nc.dram_tensor("xT", [C, T], f32r, kind="ExternalInput")
    wqk = nc.dram_tensor("wqk", [C, 2 * CL], f32r, kind="ExternalInput")
    wv = nc.dram_tensor("wv", [C, CL], f32r, kind="ExternalInput")
    wp = nc.dram_tensor("wp", [CL, C], bf16, kind="ExternalInput")
    cosP = nc.dram_tensor("cosP", [HD, T], f32, kind="ExternalInput")
    sinP = nc.dram_tensor("sinP", [HD, T], f32, kind="ExternalInput")
    rt = nc.dram_tensor("rt", [HD, HD], f32r, kind="ExternalInput")
    masks = nc.dram_tensor("masks", [4, 128, TC], bf16, kind="ExternalInput")
    ones = nc.dram_tensor("ones", [128, 128], bf16, kind="ExternalInput")
    out = nc.dram_tensor("out", [T, C], f32, kind="ExternalOutput")

    xT3 = xT.rearrange("(kt p) t -> p kt t", p=128)
    wqk3 = wqk.rearrange("(kt p) m -> p kt m", p=128)
    wv3 = wv.rearrange("(kt p) m -> p kt m", p=128)
    wp3 = wp.rearrange("(kt p) m -> p kt m", p=128)
    masks3 = masks.rearrange("d p n -> p d n")

    Exp = mybir.ActivationFunctionType.Exp

    with tile.TileContext(nc) as tc:
        mpool = tc.alloc_tile_pool(name="misc", bufs=1)
        qk_pool = tc.alloc_tile_pool(name="qkrope", bufs=1)            # 64K
        tpool = tc.alloc_tile_pool(name="trig", bufs=1, side="right")  # 16K

        rt_sb = mpool.tile([HD, HD], f32r)
        ones_sb = mpool.tile([128, 128], bf16)
        mask_sb = mpool.tile([128, 4, TC], bf16)
        cos_sb = tpool.tile([HD, T], f32)
        sin_sb = tpool.tile([HD, T], f32)

        # qkT[m] for m in 0..7: m<4 -> q head m, else k head m-4; [hd, T]
        # (rope outputs later reuse the same slots via identical tags)
        qkT = [qk_pool.tile([128, T], f32r, tag=f"qk{m}", name=f"qk{m}") for m in range(8)]

        # ---- stage A: qT/kT = W_{q,k}^T @ x^T, head-major tiles ----
        # xpool outlives wpool (reused by stage C) so it sits below it on
        # the pool stack; it is only released in the final LIFO teardown.
        xpool = tc.alloc_tile_pool(name="xt_sb", bufs=1)           # ~48K A+C
        wpool = tc.alloc_tile_pool(name="wqk_sb", bufs=1)          # 64K A
        ps1 = tc.alloc_tile_pool(name="ps_qk", bufs=1, space="PSUM")

        def x_tiles(n):
            ts = []
            for k in range(KT):
                xt = xpool.tile([128, TC], f32r, tag=f"x{k}", name=f"x{k}",
                                bufs=2 if k < 8 else 1)
                nc.sync.dma_start(out=xt, in_=xT3[:, k, n * TC:(n + 1) * TC])
                ts.append(xt)
            return ts

        # The q-half of W (wa) and the first x chunk are DMA'd interleaved,
        # the k-half (wb) after: m-groups 0..3 unlock after 6MB of DMA
        # instead of 12MB, halving the stage-A startup ramp.
        wa_t, wb_t = [], []
        x_first = []
        for k in range(KT):
            wt = wpool.tile([128, CL], f32r, tag=f"wa{k}", name=f"wa{k}")
            nc.sync.dma_start(out=wt, in_=wqk3[:, k, 0:CL])
            wa_t.append(wt)
            xt = xpool.tile([128, TC], f32r, tag=f"x{k}", name=f"x{k}",
                            bufs=2 if k < 8 else 1)
            nc.sync.dma_start(out=xt, in_=xT3[:, k, 0:TC])
            x_first.append(xt)
        for k in range(KT):
            wt = wpool.tile([128, CL], f32r, tag=f"wb{k}", name=f"wb{k}")
            nc.sync.dma_start(out=wt, in_=wqk3[:, k, CL:2 * CL])
            wb_t.append(wt)
        nc.sync.dma_start(out=rt_sb, in_=rt[:, :])
        nc.sync.dma_start(out=cos_sb, in_=cosP[:, :])
        nc.sync.dma_start(out=sin_sb, in_=sinP[:, :])
        for n in range(NT):
            x_t = x_first if n == 0 else x_tiles(n)
            for m in range(8):
                w_t = wa_t if m < 4 else wb_t
                msl = slice((m % 4) * 128, (m % 4 + 1) * 128)
                ps = ps1.tile([128, TC], f32, tag="ps_qk", name="ps_qk", bufs=8)
                for k in range(KT):
                    nc.tensor.matmul(
                        ps, w_t[k][:, msl], x_t[k],
                        start=(k == 0), stop=(k == KT - 1),
                    )
                nc.vector.tensor_copy(out=qkT[m][:, n * TC:(n + 1) * TC], in_=ps)
        wpool.release()
        ps1.release()

        # ---- stages B+C interleaved: RoPE (DVE-heavy) + v (PE-heavy) ----
        # v-phase weights and the re-streamed x chunks land in the space
        # wqk_sb/xt_sb just released; the fresh xb pool decouples the
        # re-stream DMAs from stage A's tile lifetimes so they prefetch.
        v_pool = tc.alloc_tile_pool(name="v_sb", bufs=1)           # 16K
        wvpool = tc.alloc_tile_pool(name="wv_sb", bufs=1)          # 32K
        v_t = [v_pool.tile([128, CL], bf16, tag=f"v{mt}", name=f"v{mt}")
               for mt in range(TT)]
        wv_t = []
        for k in range(KT):
            wt = wvpool.tile([128, CL], f32r, tag=f"wv{k}", name=f"wv{k}")
            nc.sync.dma_start(out=wt, in_=wv3[:, k])
            wv_t.append(wt)
        nc.sync.dma_start(out=ones_sb, in_=ones[:, :])
        nc.sync.dma_start(out=mask_sb, in_=masks3)

        rope = [None] * 8
        rtmp = tc.alloc_tile_pool(name="rope_tmp", bufs=4)
        psr = tc.alloc_tile_pool(name="ps_rot", bufs=2, space="PSUM")
        ps2 = tc.alloc_tile_pool(name="ps_v", bufs=2, space="PSUM")
        m_order = (0, 4, 1, 5, 2, 6, 3, 7)

        # RoPE is emitted piecewise (one 512-col chunk at a time) between
        # v-matmul groups, so the PE keeps streaming v while the DVE
        # digests the elementwise work; v PSUM evacuation runs on the
        # otherwise-idle ACT engine.
        rope_state = {}  # m -> list of (t1, t2)

        def rope_piece(m, nn):
            sl = slice(nn * TC, (nn + 1) * TC)
            ps = psr.tile([128, TC], f32, tag="ps_rot", name="ps_rot")
            nc.tensor.matmul(ps, rt_sb, qkT[m][:, sl], start=True, stop=True)
            t1 = rtmp.tile([128, TC], f32, tag="t1", name="t1")
            t2 = rtmp.tile([128, TC], f32, tag="t2", name="t2")
            nc.vector.tensor_mul(t1, ps, sin_sb[:, sl])
            nc.vector.tensor_mul(t2, qkT[m][:, sl], cos_sb[:, sl])
            rope_state.setdefault(m, []).append((t1, t2))
            if nn == NT - 1:
                # all reads of qkT[m] issued; now write into its slot
                ro = qk_pool.tile([128, T], f32r, tag=f"qk{m}", name=f"rope{m}")
                for k in range(NT):
                    ksl = slice(k * TC, (k + 1) * TC)
                    ta, tb = rope_state[m][k]
                    nc.vector.tensor_add(ro[:, ksl], ta, tb)
                rope[m] = ro

        rope_pieces = [(m, nn) for m in m_order for nn in range(NT)]
        rp = 0
        # chunk 3's x tiles are still resident from stage A — consume them
        # first (no DMA wait), re-streaming chunks 0..2 behind them.
        for n, x_t in [(3, x_t), (0, None), (1, None), (2, None)]:
            if x_t is None:
                x_t = x_tiles(n)
            for sub in range(4):
                mt = 4 * n + sub
                ps = ps2.tile([128, CL], f32, tag="ps_v", name="ps_v")
                for k in range(KT):
                    nc.tensor.matmul(
                        ps, x_t[k][:, sub * 128:(sub + 1) * 128], wv_t[k],
                        start=(k == 0), stop=(k == KT - 1),
                    )
                nc.scalar.copy(out=v_t[mt], in_=ps)
                for _ in range(2):
                    if rp < len(rope_pieces):
                        rope_piece(*rope_pieces[rp])
                        rp += 1
        while rp < len(rope_pieces):
            rope_piece(*rope_pieces[rp])
            rp += 1
        rtmp.release()
        wvpool.release()
        ps2.release()
        psr.release()
        tpool.release()
        # output-projection weights for stage E land in the space wv_sb
        # just released (first needed ~25us into stage D)
        wppool = tc.alloc_tile_pool(name="wp_sb", bufs=1)          # 16K
        wp_t = []
        for hk in range(HPC):
            wt = wppool.tile([128, C], bf16, tag=f"wp{hk}", name=f"wp{hk}")
            nc.sync.dma_start(out=wt, in_=wp3[:, hk])
            wp_t.append(wt)

        # ---- stage D (I-outer, h-inner) with stage E one chunk behind ----
        y_pool = tc.alloc_tile_pool(name="yT_sb", bufs=1)          # 16K
        yT = [y_pool.tile([128, T], bf16, tag=f"yT{h}", name=f"yT{h}")
              for h in range(HPC)]

        pp_pool = tc.alloc_tile_pool(name="pp", bufs=4)
        sm_pool = tc.alloc_tile_pool(name="small", bufs=2)
        opool = tc.alloc_tile_pool(name="ostage", bufs=3)
        # 8 PSUM banks: 3 score (2-deep exp lookahead) + 2 y + 1 d + 2
        # proj. ps_d single-buffered: the reciprocal reads it within ~1us
        # and the next head's first d-matmul lands later than that. The
        # d-matmul's stationary operand is an all-ones [128,128] matrix, so
        # d lands replicated across all partitions and the reciprocal feeds
        # the normalize multiply directly (no partition broadcast).
        ps_s = tc.alloc_tile_pool(name="ps_s", bufs=3, space="PSUM")
        ps_y = tc.alloc_tile_pool(name="ps_y", bufs=2, space="PSUM")
        ps_d = tc.alloc_tile_pool(name="ps_d", bufs=1, space="PSUM")
        ps_o = tc.alloc_tile_pool(name="ps_o", bufs=2, space="PSUM")

        e_queue = []

        def emit_e_group():
            mt, nn = e_queue.pop(0)
            msl = slice(mt * 128, (mt + 1) * 128)
            ps = ps_o.tile([128, TC], f32, tag="o", name="o_ps")
            for hk in range(HPC):
                nc.tensor.matmul(
                    ps, yT[hk][:, msl], wp_t[hk][:, nn * TC:(nn + 1) * TC],
                    start=(hk == 0), stop=(hk == HPC - 1),
                )
            ot = opool.tile([128, TC], f32, tag="ot", name="ot")
            nc.vector.tensor_copy(out=ot, in_=ps)
            nc.sync.dma_start(out=out[msl, nn * TC:(nn + 1) * TC], in_=ot)

        # Stage D processes key tiles in PAIRS with a two-pair S lookahead:
        # the PE stream is [S S] [E] [PV PV] [d d] per pair. Consecutive
        # same-bank matmuls (PV,PV / d,d) avoid the ~107ns PSUM
        # bank-switch drain penalty, and the 4-tile-deep exp pipeline plus
        # the woven stage-E group give the Scalar engine's exp a full
        # pair-period of slack before PV consumes it.
        for I in range(NT):
            isl = slice(I * TC, (I + 1) * TC)
            jl = _JLISTS[I]
            pairs = [jl[i:i + 2] for i in range(0, len(jl), 2)]
            for h in range(HPC):
                q_h = rope[h]
                k_h = rope[4 + h]
                y_ps = ps_y.tile([128, TC], f32, tag="y", name="y_ps")
                d_ps = ps_d.tile([128, TC], f32, tag="d", name="d_ps")

                def emit_s_pair(pair):
                    pps = []
                    for (J, dm) in pair:
                        s_ps = ps_s.tile([128, TC], f32, tag="s", name="s_ps")
                        nc.tensor.matmul(
                            s_ps, k_h[:, J * 128:(J + 1) * 128],
                            q_h[:, isl], start=True, stop=True,
                        )
                        pp = pp_pool.tile([128, TC], bf16, tag="pp", name="pp",
                                          bufs=6)
                        nc.scalar.activation(out=pp, in_=s_ps, func=Exp,
                                             scale=SCALE)
                        pps.append(pp)
                    return pps

                pending = [emit_s_pair(pairs[0])]
                if e_queue:
                    emit_e_group()
                if len(pairs) > 1:
                    pending.append(emit_s_pair(pairs[1]))
                jidx = 0
                for t, pair in enumerate(pairs):
                    pps = pending.pop(0)
                    if t + 2 < len(pairs):
                        pending.append(emit_s_pair(pairs[t + 2]))
                    if e_queue and (t % 2 == 1 or I == NT - 1):
                        emit_e_group()
                    for pi, (J, dm) in enumerate(pair):
                        if dm is not None:
                            ppm = pp_pool.tile([128, TC], bf16, tag="ppm",
                                               name="ppm", bufs=2)
                            nc.vector.tensor_mul(ppm, pps[pi], mask_sb[:, dm])
                            pps[pi] = ppm
                    for pi, (J, dm) in enumerate(pair):
                        nc.tensor.matmul(
                            y_ps, v_t[J][:, h * 128:(h + 1) * 128], pps[pi],
                            start=(jidx + pi == 0),
                            stop=(jidx + pi == len(jl) - 1),
                        )
                    for pi, (J, dm) in enumerate(pair):
                        nc.tensor.matmul(d_ps, ones_sb, pps[pi],
                                         start=(jidx + pi == 0),
                                         stop=(jidx + pi == len(jl) - 1))
                    jidx += len(pair)
                recipB = sm_pool.tile([128, TC], f32, tag="recipB",
                                      name="recipB")
                nc.vector.reciprocal_approx_fast(out=recipB, in_=d_ps)
                nc.vector.tensor_mul(yT[h][:, isl], y_ps, recipB)
            e_queue.extend((4 * I + ml, nn) for ml in range(4) for nn in range(NT))
        while e_queue:
            emit_e_group()

        for p in (opool, sm_pool, pp_pool, y_pool, wppool, v_pool,
                  xpool, qk_pool, mpool, ps_o, ps_d, ps_y, ps_s):
            p.release()
    nc.compile()
    return nc


def _host_prep(x, w_qkv, w_proj, freqs_cis):
    """Build per-core input maps (slicing + layout prep only)."""
    import ml_dtypes

    bf16 = ml_dtypes.bfloat16
    x = np.asarray(x, dtype=np.float32)
    w_qkv = np.asarray(w_qkv, dtype=np.float32)
    w_proj = np.asarray(w_proj, dtype=np.float32)
    fc = np.asarray(freqs_cis, dtype=np.float32)

    xTb = [np.ascontiguousarray(x[b].T) for b in range(B)]

    cos = fc[:, :, 0].T  # [64, T]
    sin = fc[:, :, 1].T
    cosP = np.repeat(cos, 2, axis=0).astype(np.float32)  # [128, T]
    sinP = np.repeat(sin, 2, axis=0).astype(np.float32)

    rt = np.zeros((HD, HD), dtype=np.float32)
    for d in range(HD // 2):
        rt[2 * d, 2 * d + 1] = 1.0
        rt[2 * d + 1, 2 * d] = -1.0

    masks = np.zeros((4, 128, TC), dtype=np.float32)
    ii = np.arange(TC)[None, :]
    jj = np.arange(128)[:, None]
    for d in range(4):
        masks[d] = (ii >= jj + 128 * d).astype(np.float32)
    masks = masks.astype(bf16)

    ones = np.ones((128, 128), dtype=bf16)

    in_maps = []
    for core in range(8):
        b = core // 4
        g = core % 4
        qc = np.ascontiguousarray(w_qkv[:, 512 * g: 512 * (g + 1)])
        kc = np.ascontiguousarray(w_qkv[:, 2048 + 512 * g: 2048 + 512 * (g + 1)])
        vc = np.ascontiguousarray(w_qkv[:, 4096 + 512 * g: 4096 + 512 * (g + 1)])
        wqk_c = np.concatenate([qc, kc], axis=1)
        wp_c = np.ascontiguousarray(
            w_proj[512 * g: 512 * (g + 1), :]).astype(bf16)
        in_maps.append({
            "xT": xTb[b],
            "wqk": wqk_c,
            "wv": vc,
            "wp": wp_c,
            "cosP": cosP,
            "sinP": sinP,
            "rt": rt,
            "masks": masks,
            "ones": ones,
        })
    return in_maps


def _get_nc():
    if "nc" not in _CACHE:
        _CACHE["nc"] = _build_nc()
    return _CACHE["nc"]


def kernel(x, w_qkv, w_proj, freqs_cis, attn_mask, _trace=False):
    from concourse.bass_utils import run_bass_kernel_spmd

    in_maps = _host_prep(x, w_qkv, w_proj, freqs_cis)
    nc = _get_nc()
    res = run_bass_kernel_spmd(
        nc, in_maps, core_ids=list(range(8)), trace=_trace,
    )
    outs = [r["out"].astype(np.float64) for r in res.results]
    full = np.stack([
        outs[0] + outs[1] + outs[2] + outs[3],
        outs[4] + outs[5] + outs[6] + outs[7],
    ]).astype(np.float32)
    if _trace:
        kernel._last_results = res
    return full


# revision 30
# speedup vs baseline: 1.6136x; 1.0097x over previous
"""Trainium2 Bass kernel for prefix-LM CausalSelfAttention.

Problem: B=2, T=2048, C=2048, H=16 heads (hd=128), prefix-LM mask
(bidirectional over first half, causal after), RoPE on q/k.

Sharding over 8 cores: data-parallel on batch (2) x tensor-parallel on
heads (4 heads per core). Each core computes a partial output projection
(its heads' contribution); partials are summed on host.

Per-core dataflow:
  A. qT/kT = W^T @ x^T    [hd*4, T] head-major tiles (f32r, full PE rate);
     the q-half of W is DMA'd interleaved with the first x chunk so the
     PE starts before the whole weight load lands.
  B. RoPE via pair-swap permutation matmul + DVE combine (f32r)
  C. v = x @ Wv           [T, hd*4] natural layout, cast to bf16
     B and C are emitted interleaved piecewise so the PE computes v while
     the DVE does RoPE's elementwise work (PE would otherwise idle ~50us);
     v PSUM evacuation runs on the otherwise-idle ACT engine.
  D. Per 512-query chunk I (outer), per head h (inner), over unmasked
     128-key tiles J, processed in PAIRS ([S S][E][PV PV][d d] in the PE
     stream — consecutive same-PSUM-bank matmuls avoid the ~107ns
     bank-switch drain penalty, and the 4-tile-deep S/exp lookahead hides
     the cross-engine exp latency):
       S'[J] = k_rope[:,J]^T-tile x q_rope[:,I]    (f32r, scores transposed)
       P'[J] = exp(S' * 1/sqrt(hd))                (ACT, PSUM->SBUF, bf16)
       mask-multiply for diagonal-crossing tiles only (DVE bf16)
       y_psum += v[J,h]^T-as-lhsT x P'[J]          (bf16 PV, out y^T [hd,i])
       d_psum += ones128^T x P'[J]  (all-ones [128,128] stationary: the
         denominator lands replicated across partitions, so the DVE
         fast-approx reciprocal feeds the normalize multiply directly)
     y^T[:, I] = y_psum * recip(d)  (the same bf16 P' feeds PV and d so
     rounding cancels in the ratio)
  E. partial_out = y^T-as-lhsT x Wp (bf16) accumulated over the 4 heads,
     woven into stage D one chunk behind (chunk I's projection groups run
     between chunk I+1's attention pairs); PSUM evacuation on DVE (keeps
     ACT free for exp, the stage-D co-bottleneck).

Fully-masked key tiles are skipped (structural sparsity: 44/64 tiles/head).
"""
import math

import numpy as np

N_HEAD = 16
B = 2
T = 2048
C = 2048
HD = 128
HPC = 4          # heads per core
CL = HPC * HD    # local C = 512
TC = 512         # chunk width (matmul moving free dim / psum bank)
NT = T // TC     # 4 chunks
KT = C // 128    # 16 contraction tiles over C
TT = T // 128    # 16 T tiles
SCALE = 1.0 / math.sqrt(HD)

# Per query-chunk I: list of (J, mask_idx) key tiles to compute.
# mask_idx is None for fully-allowed tiles, else 0..3 selecting the
# static diagonal pattern mask[d][jj, ii] = (ii >= jj + 128*d).
_JLISTS = {
    0: [(j, None) for j in range(8)],
    1: [(j, None) for j in range(8)],
    2: [(j, None) for j in range(8)] + [(8 + d, d) for d in range(4)],
    3: [(j, None) for j in range(12)] + [(12 + d, d) for d in range(4)],
}

_CACHE = {}


def _build_nc():
    import concourse.tile as tile
    import concourse.mybir as mybir
    from concourse import bacc

    f32 = mybir.dt.float32
    f32r = mybir.dt.float32r
    bf16 = mybir.dt.bfloat16

    nc = bacc.Bacc(None, target_bir_lowering=False)

    xT = nc.dram_tensor("xT", [C, T], f32r, kind="ExternalInput")
    wqk = nc.dram_tensor("wqk", [C, 2 * CL], f32r, kind="ExternalInput")
    wv = nc.dram_tensor("wv", [C, CL], f32r, kind="ExternalInput")
    wp = nc.dram_tensor("wp", [CL, C], bf16, kind="ExternalInput")
    cosP = nc.dram_tensor("cosP", [HD, T], f32, kind="ExternalInput")
    sinP = nc.dram_tensor("sinP", [HD, T], f32, kind="ExternalInput")
    rt = nc.dram_tensor("rt", [HD, HD], f32r, kind="ExternalInput")
    masks = nc.dram_tensor("masks", [4, 128, TC], bf16, kind="ExternalInput")
    ones = nc.dram_tensor("ones", [128, 128], bf16, kind="ExternalInput")
    out = nc.dram_tensor("out", [T, C], f32, kind="ExternalOutput")

    xT3 = xT.rearrange("(kt p) t -> p kt t", p=128)
    wqk3 = wqk.rearrange("(kt p) m -> p kt m", p=128)
    wv3 = wv.rearrange("(kt p) m -> p kt m", p=128)
    wp3 = wp.rearrange("(kt p) m -> p kt m", p=128)
    masks3 = masks.rearrange("d p n -> p d n")

    Exp = mybir.ActivationFunctionType.Exp

    with tile.TileContext(nc) as tc:
        mpool = tc.alloc_tile_pool(name="misc", bufs=1)
        qk_pool = tc.alloc_tile_pool(name="qkrope", bufs=1)            # 64K
        tpool = tc.alloc_tile_pool(name="trig", bufs=1, side="right")  # 16K

        rt_sb = mpool.tile([HD, HD], f32r)
        ones_sb = mpool.tile([128, 128], bf16)
        mask_sb = mpool.tile([128, 4, TC], bf16)
        cos_sb = tpool.tile([HD, T], f32)
        sin_sb = tpool.tile([HD, T], f32)

        # qkT[m] for m in 0..7: m<4 -> q head m, else k head m-4; [hd, T]
        # (rope outputs later reuse the same slots via identical tags)
        qkT = [qk_pool.tile([128, T], f32r, tag=f"qk{m}", name=f"qk{m}") for m in range(8)]

        # ---- stage A: qT/kT = W_{q,k}^T @ x^T, head-major tiles ----
        # xpool outlives wpool (reused by stage C) so it sits below it on
        # the pool stack; it is only released in the final LIFO teardown.
        xpool = tc.alloc_tile_pool(name="xt_sb", bufs=1)           # ~48K A+C
        wpool = tc.alloc_tile_pool(name="wqk_sb", bufs=1)          # 64K A
        ps1 = tc.alloc_tile_pool(name="ps_qk", bufs=1, space="PSUM")

        def x_tiles(n):
            ts = []
            for k in range(KT):
                xt = xpool.tile([128, TC], f32r, tag=f"x{k}", name=f"x{k}",
                                bufs=2 if k < 8 else 1)
                nc.sync.dma_start(out=xt, in_=xT3[:, k, n * TC:(n + 1) * TC])
                ts.append(xt)
            return ts

        # The q-half of W (wa) and the first x chunk are DMA'd interleaved,
        # the k-half (wb) after: m-groups 0..3 unlock after 6MB of DMA
        # instead of 12MB, halving the stage-A startup ramp.
        wa_t, wb_t = [], []
        x_first = []
        for k in range(KT):
            wt = wpool.tile([128, CL], f32r, tag=f"wa{k}", name=f"wa{k}")
            nc.sync.dma_start(out=wt, in_=wqk3[:, k, 0:CL])
            wa_t.append(wt)
            xt = xpool.tile([128, TC], f32r, tag=f"x{k}", name=f"x{k}",
                            bufs=2 if k < 8 else 1)
            nc.sync.dma_start(out=xt, in_=xT3[:, k, 0:TC])
            x_first.append(xt)
        for k in range(KT):
            wt = wpool.tile([128, CL], f32r, tag=f"wb{k}", name=f"wb{k}")
            nc.sync.dma_start(out=wt, in_=wqk3[:, k, CL:2 * CL])
            wb_t.append(wt)
        nc.sync.dma_start(out=rt_sb, in_=rt[:, :])
        nc.sync.dma_start(out=cos_sb, in_=cosP[:, :])
        nc.sync.dma_start(out=sin_sb, in_=sinP[:, :])
        for n in range(NT):
            x_t = x_first if n == 0 else x_tiles(n)
            for m in range(8):
                w_t = wa_t if m < 4 else wb_t
                msl = slice((m % 4) * 128, (m % 4 + 1) * 128)
                ps = ps1.tile([128, TC], f32, tag="ps_qk", name="ps_qk", bufs=8)
                for k in range(KT):
                    nc.tensor.matmul(
                        ps, w_t[k][:, msl], x_t[k],
                        start=(k == 0), stop=(k == KT - 1),
                    )
                nc.vector.tensor_copy(out=qkT[m][:, n * TC:(n + 1) * TC], in_=ps)
        wpool.release()
        ps1.release()

        # ---- stages B+C interleaved: RoPE (DVE-heavy) + v (PE-heavy) ----
        # v-phase weights and the re-streamed x chunks land in the space
        # wqk_sb/xt_sb just released; the fresh xb pool decouples the
        # re-stream DMAs from stage A's tile lifetimes so they prefetch.
        v_pool = tc.alloc_tile_pool(name="v_sb", bufs=1)           # 16K
        wvpool = tc.alloc_tile_pool(name="wv_sb", bufs=1)          # 32K
        v_t = [v_pool.tile([128, CL], bf16, tag=f"v{mt}", name=f"v{mt}")
               for mt in range(TT)]
        wv_t = []
        for k in range(KT):
            wt = wvpool.tile([128, CL], f32r, tag=f"wv{k}", name=f"wv{k}")
            nc.sync.dma_start(out=wt, in_=wv3[:, k])
            wv_t.append(wt)
        nc.sync.dma_start(out=ones_sb, in_=ones[:, :])
        nc.sync.dma_start(out=mask_sb, in_=masks3)

        rope = [None] * 8
        rtmp = tc.alloc_tile_pool(name="rope_tmp", bufs=4)
        psr = tc.alloc_tile_pool(name="ps_rot", bufs=2, space="PSUM")
        ps2 = tc.alloc_tile_pool(name="ps_v", bufs=2, space="PSUM")
        m_order = (0, 4, 1, 5, 2, 6, 3, 7)

        # RoPE is emitted piecewise (one 512-col chunk at a time) between
        # v-matmul groups, so the PE keeps streaming v while the DVE
        # digests the elementwise work; v PSUM evacuation runs on the
        # otherwise-idle ACT engine.
        rope_state = {}  # m -> list of (t1, t2)

        def rope_piece(m, nn):
            sl = slice(nn * TC, (nn + 1) * TC)
            ps = psr.tile([128, TC], f32, tag="ps_rot", name="ps_rot")
            nc.tensor.matmul(ps, rt_sb, qkT[m][:, sl], start=True, stop=True)
            t1 = rtmp.tile([128, TC], f32, tag="t1", name="t1")
            t2 = rtmp.tile([128, TC], f32, tag="t2", name="t2")
            nc.vector.tensor_mul(t1, ps, sin_sb[:, sl])
            nc.vector.tensor_mul(t2, qkT[m][:, sl], cos_sb[:, sl])
            rope_state.setdefault(m, []).append((t1, t2))
            if nn == NT - 1:
                # all reads of qkT[m] issued; now write into its slot
                ro = qk_pool.tile([128, T], f32r, tag=f"qk{m}", name=f"rope{m}")
                for k in range(NT):
                    ksl = slice(k * TC, (k + 1) * TC)
                    ta, tb = rope_state[m][k]
                    nc.vector.tensor_add(ro[:, ksl], ta, tb)
                rope[m] = ro

        rope_pieces = [(m, nn) for m in m_order for nn in range(NT)]
        rp = 0
        # chunk 3's x tiles are still resident from stage A — consume them
        # first (no DMA wait), re-streaming chunks 0..2 behind them.
        for n, x_t in [(3, x_t), (0, None), (1, None), (2, None)]:
            if x_t is None:
                x_t = x_tiles(n)
            for sub in range(4):
                mt = 4 * n + sub
                ps = ps2.tile([128, CL], f32, tag="ps_v", name="ps_v")
                for k in range(KT):
                    nc.tensor.matmul(
                        ps, x_t[k][:, sub * 128:(sub + 1) * 128], wv_t[k],
                        start=(k == 0), stop=(k == KT - 1),
                    )
                nc.scalar.copy(out=v_t[mt], in_=ps)
                for _ in range(2):
                    if rp < len(rope_pieces):
                        rope_piece(*rope_pieces[rp])
                        rp += 1
        while rp < len(rope_pieces):
            rope_piece(*rope_pieces[rp])
            rp += 1
        rtmp.release()
        wvpool.release()
        ps2.release()
        psr.release()
        tpool.release()
        # output-projection weights for stage E land in the space wv_sb
        # just released (first needed ~25us into stage D)
        wppool = tc.alloc_tile_pool(name="wp_sb", bufs=1)          # 16K
        wp_t = []
        for hk in range(HPC):
            wt = wppool.tile([128, C], bf16, tag=f"wp{hk}", name=f"wp{hk}")
            nc.sync.dma_start(out=wt, in_=wp3[:, hk])
            wp_t.append(wt)

        # ---- stage D (I-outer, h-inner) with stage E one chunk behind ----
        y_pool = tc.alloc_tile_pool(name="yT_sb", bufs=1)          # 16K
        yT = [y_pool.tile([128, T], bf16, tag=f"yT{h}", name=f"yT{h}")
              for h in range(HPC)]

        pp_pool = tc.alloc_tile_pool(name="pp", bufs=4)
        sm_pool = tc.alloc_tile_pool(name="small", bufs=2)
        opool = tc.alloc_tile_pool(name="ostage", bufs=3)
        # 8 PSUM banks: 3 score (2-deep exp lookahead) + 2 y + 1 d + 2
        # proj. ps_d single-buffered: the reciprocal reads it within ~1us
        # and the next head's first d-matmul lands later than that. The
        # d-matmul's stationary operand is an all-ones [128,128] matrix, so
        # d lands replicated across all partitions and the reciprocal feeds
        # the normalize multiply directly (no partition broadcast).
        ps_s = tc.alloc_tile_pool(name="ps_s", bufs=3, space="PSUM")
        ps_y = tc.alloc_tile_pool(name="ps_y", bufs=2, space="PSUM")
        ps_d = tc.alloc_tile_pool(name="ps_d", bufs=1, space="PSUM")
        ps_o = tc.alloc_tile_pool(name="ps_o", bufs=2, space="PSUM")

        e_queue = []

        def emit_e_group():
            mt, nn = e_queue.pop(0)
            msl = slice(mt * 128, (mt + 1) * 128)
            ps = ps_o.tile([128, TC], f32, tag="o", name="o_ps")
            for hk in range(HPC):
                nc.tensor.matmul(
                    ps, yT[hk][:, msl], wp_t[hk][:, nn * TC:(nn + 1) * TC],
                    start=(hk == 0), stop=(hk == HPC - 1),
                )
            ot = opool.tile([128, TC], f32, tag="ot", name="ot")
            nc.vector.tensor_copy(out=ot, in_=ps)
            nc.sync.dma_start(out=out[msl, nn * TC:(nn + 1) * TC], in_=ot)

        # Stage D processes key tiles in PAIRS with a two-pair S lookahead:
        # the PE stream is [S S] [E] [PV PV] [d d] per pair. Consecutive
        # same-bank matmuls (PV,PV / d,d) avoid the ~107ns PSUM
        # bank-switch drain penalty, and the 4-tile-deep exp pipeline plus
        # the woven stage-E group give the Scalar engine's exp a full
        # pair-period of slack before PV consumes it.
        for I in range(NT):
            isl = slice(I * TC, (I + 1) * TC)
            jl = _JLISTS[I]
            pairs = [jl[i:i + 2] for i in range(0, len(jl), 2)]
            for h in range(HPC):
                q_h = rope[h]
                k_h = rope[4 + h]
                y_ps = ps_y.tile([128, TC], f32, tag="y", name="y_ps")
                d_ps = ps_d.tile([128, TC], f32, tag="d", name="d_ps")

                def emit_s_pair(pair):
                    pps = []
                    for (J, dm) in pair:
                        s_ps = ps_s.tile([128, TC], f32, tag="s", name="s_ps")
                        nc.tensor.matmul(
                            s_ps, k_h[:, J * 128:(J + 1) * 128],
                            q_h[:, isl], start=True, stop=True,
                        )
                        pp = pp_pool.tile([128, TC], bf16, tag="pp", name="pp",
                                          bufs=6)
                        nc.scalar.activation(out=pp, in_=s_ps, func=Exp,
                                             scale=SCALE)
                        pps.append(pp)
                    return pps

                pending = [emit_s_pair(pairs[0])]
                if e_queue:
                    emit_e_group()
                if len(pairs) > 1:
                    pending.append(emit_s_pair(pairs[1]))
                jidx = 0
                for t, pair in enumerate(pairs):
                    pps = pending.pop(0)
                    if t + 2 < len(pairs):
                        pending.append(emit_s_pair(pairs[t + 2]))
                    if e_queue and (t % 2 == 1 or I == NT - 1):
                        emit_e_group()
                    for pi, (J, dm) in enumerate(pair):
                        if dm is not None:
                            ppm = pp_pool.tile([128, TC], bf16, tag="ppm",
                                               name="ppm", bufs=2)
                            nc.vector.tensor_mul(ppm, pps[pi], mask_sb[:, dm])
                            pps[pi] = ppm
                    for pi, (J, dm) in enumerate(pair):
                        nc.tensor.matmul(
                            y_ps, v_t[J][:, h * 128:(h + 1) * 128], pps[pi],
                            start=(jidx + pi == 0),
                            stop=(jidx + pi == len(jl) - 1),
                        )
                    for pi, (J, dm) in enumerate(pair):
                        nc.tensor.matmul(d_ps, ones_sb, pps[pi],
                                         start=(jidx + pi == 0),
                                         stop=(jidx + pi == len(jl) - 1))
                    jidx += len(pair)
                recipB = sm_pool.tile([128, TC], f32, tag="recipB",
                                      name="recipB")
                nc.vector.reciprocal_approx_fast(out=recipB, in_=d_ps)
                nc.vector.tensor_mul(yT[h][:, isl], y_ps, recipB)
            e_queue.extend((4 * I + ml, nn) for ml in range(4) for nn in range(NT))
        while e_queue:
            emit_e_group()

        for p in (opool, sm_pool, pp_pool, y_pool, wppool, v_pool,
                  xpool, qk_pool, mpool, ps_o, ps_d, ps_y, ps_s):
            p.release()
    nc.compile()
    return nc


def _host_prep(x, w_qkv, w_proj, freqs_cis):
    """Build per-core input maps (slicing + layout prep only)."""
    import ml_dtypes

    bf16 = ml_dtypes.bfloat16
    x = np.asarray(x, dtype=np.float32)
    w_qkv = np.asarray(w_qkv, dtype=np.float32)
    w_proj = np.asarray(w_proj, dtype=np.float32)
    fc = np.asarray(freqs_cis, dtype=np.float32)

    xTb = [np.ascontiguousarray(x[b].T) for b in range(B)]

    cos = fc[:, :, 0].T  # [64, T]
    sin = fc[:, :, 1].T
    cosP = np.repeat(cos, 2, axis=0).astype(np.float32)  # [128, T]
    sinP = np.repeat(sin, 2, axis=0).astype(np.float32)

    rt = np.zeros((HD, HD), dtype=np.float32)
    for d in range(HD // 2):
        rt[2 * d, 2 * d + 1] = 1.0
        rt[2 * d + 1, 2 * d] = -1.0

    masks = np.zeros((4, 128, TC), dtype=np.float32)
    ii = np.arange(TC)[None, :]
    jj = np.arange(128)[:, None]
    for d in range(4):
        masks[d] = (ii >= jj + 128 * d).astype(np.float32)
    masks = masks.astype(bf16)

    ones = np.ones((128, 128), dtype=bf16)

    in_maps = []
    for core in range(8):
        b = core // 4
        g = core % 4
        qc = np.ascontiguousarray(w_qkv[:, 512 * g: 512 * (g + 1)])
        kc = np.ascontiguousarray(w_qkv[:, 2048 + 512 * g: 2048 + 512 * (g + 1)])
        vc = np.ascontiguousarray(w_qkv[:, 4096 + 512 * g: 4096 + 512 * (g + 1)])
        wqk_c = np.concatenate([qc, kc], axis=1)
        wp_c = np.ascontiguousarray(
            w_proj[512 * g: 512 * (g + 1), :]).astype(bf16)
        in_maps.append({
            "xT": xTb[b],
            "wqk": wqk_c,
            "wv": vc,
            "wp": wp_c,
            "cosP": cosP,
            "sinP": sinP,
            "rt": rt,
            "masks": masks,
            "ones": ones,
        })
    return in_maps


def _get_nc():
    if "nc" not in _CACHE:
        _CACHE["nc"] = _build_nc()
    return _CACHE["nc"]


def kernel(x, w_qkv, w_proj, freqs_cis, attn_mask, _trace=False):
    from concourse.bass_utils import run_bass_kernel_spmd

    in_maps = _host_prep(x, w_qkv, w_proj, freqs_cis)
    nc = _get_nc()
    res = run_bass_kernel_spmd(
        nc, in_maps, core_ids=list(range(8)), trace=_trace,
    )
    outs = [r["out"].astype(np.float64) for r in res.results]
    full = np.stack([
        outs[0] + outs[1] + outs[2] + outs[3],
        outs[4] + outs[5] + outs[6] + outs[7],
    ]).astype(np.float32)
    if _trace:
        kernel._last_results = res
    return full
